# revision 6
# baseline (speedup 1.0000x reference)
"""GQA attention kernel for 8 trn2 NeuronCores.

Sharding: core c in 0..7 -> batch b = c//4, KV group g = c%4 (4 Q heads,
1 KV head per core). Tensor-parallel on Wq/Wk/Wv columns and Wo rows;
host sums the 4 partial outputs per batch.

Layout strategy: all device tensors are transposed ([feature, seq]) so that
Q^T/K^T fall directly out of the projection matmuls, S^T = (K^T-slice)^T @ Q^T
keeps softmax's reduction on the PARTITION axis (summed with a ones-matmul on
the PE), and V (natural [k,d]) is recovered with PE transposes. All matmuls
run in float32r (full PE rate at moving-dim 512, ~1e-4 rel err).
"""
import sys
sys.path.insert(0, "/opt/trn_rl_repo")
import math
import numpy as np

B, L, D = 2, 2048, 2048
H, HKV, HD = 16, 4, 128
BASE = 10000.0
NT = L // 128      # 16 seq tiles of 128
NCH = L // 512     # 4 seq chunks of 512
NH = H // HKV      # 4 heads per core
SCALE = 1.0 / math.sqrt(HD)

_compiled = None


def _build():
    from concourse import bacc, tile, mybir

    f32, f32r = mybir.dt.float32, mybir.dt.float32r
    Exp = mybir.ActivationFunctionType.Exp
    mult, add = mybir.AluOpType.mult, mybir.AluOpType.add

    nc = bacc.Bacc("TRN2", target_bir_lowering=False, debug=False,
                   enable_asserts=True, num_devices=8)

    xT_d = nc.dram_tensor("xT", [D, L], f32, kind="ExternalInput")
    wq_d = nc.dram_tensor("wq", [D, NH * HD], f32, kind="ExternalInput")
    wk_d = nc.dram_tensor("wk", [D, HD], f32, kind="ExternalInput")
    wv_d = nc.dram_tensor("wv", [D, HD], f32, kind="ExternalInput")
    wo_d = nc.dram_tensor("wo", [NH * HD, D], f32, kind="ExternalInput")
    cos_d = nc.dram_tensor("cosT", [HD, L], f32, kind="ExternalInput")
    sin_d = nc.dram_tensor("sinT", [HD, L], f32, kind="ExternalInput")
    shp_d = nc.dram_tensor("shiftP", [HD, HD], f32, kind="ExternalInput")
    idn_d = nc.dram_tensor("ident", [128, 128], f32, kind="ExternalInput")
    onc_d = nc.dram_tensor("onescol", [128, 1], f32, kind="ExternalInput")
    onr_d = nc.dram_tensor("onesrow", [1, 128], f32, kind="ExternalInput")
    y_d = nc.dram_tensor("y", [L, D], f32, kind="ExternalOutput")

    with tile.TileContext(nc) as tc, \
         nc.allow_low_precision(reason="float32r is 4-byte storage; matmul "
                                "inputs round to fp32r by design"):
        with tc.tile_pool(name="persist", bufs=1) as pp:
            # long-lived tiles (unique tag per tile => exact allocation)
            shp = pp.tile([HD, HD], f32r, tag="shp", name="shp")
            idn = pp.tile([128, 128], f32, tag="idn", name="idn")
            onc = pp.tile([128, 1], f32r, tag="onc", name="onc")
            onr = pp.tile([1, 128], f32r, tag="onr", name="onr")
            nc.sync.dma_start(shp[:], shp_d[:].bitcast(f32r))
            nc.sync.dma_start(idn[:], idn_d[:])
            nc.sync.dma_start(onc[:], onc_d[:].bitcast(f32r))
            nc.sync.dma_start(onr[:], onr_d[:].bitcast(f32r))

            qt = [[pp.tile([HD, 512], f32r, tag=f"qt{h}_{n}", name=f"qt{h}_{n}") for n in range(NCH)]
                  for h in range(NH)]
            kt = [pp.tile([HD, 512], f32r, tag=f"kt{n}", name=f"kt{n}") for n in range(NCH)]
            vn = [pp.tile([128, HD], f32r, tag=f"vn{t}", name=f"vn{t}") for t in range(NT)]
            ot = [[pp.tile([HD, 512], f32r, tag=f"ot{h}_{n}", name=f"ot{h}_{n}") for n in range(NCH)]
                  for h in range(NH)]
            wo = [pp.tile([HD, L], f32r, tag=f"wo{h}", name=f"wo{h}") for h in range(NH)]
            for h in range(NH):
                nc.sync.dma_start(wo[h][:], wo_d[h * HD:(h + 1) * HD, :].bitcast(f32r))

            # ---------------- Phase A: projections + RoPE + V transpose ----
            with tc.tile_pool(name="aw", bufs=1) as aw, \
                 tc.tile_pool(name="ax", bufs=3) as ax, \
                 tc.tile_pool(name="atmp", bufs=3) as at, \
                 tc.tile_pool(name="apsum", bufs=1, space="PSUM") as aps:
                wq = [aw.tile([128, NH * HD], f32r, tag=f"wq{c}", name=f"wq{c}") for c in range(NT)]
                wk = [aw.tile([128, HD], f32r, tag=f"wk{c}", name=f"wk{c}") for c in range(NT)]
                wv = [aw.tile([128, HD], f32r, tag=f"wv{c}", name=f"wv{c}") for c in range(NT)]
                cosT = aw.tile([HD, L], f32, tag="cos", name="cos")
                sinT = aw.tile([HD, L], f32, tag="sin", name="sin")
                nc.sync.dma_start(cosT[:], cos_d[:])
                nc.sync.dma_start(sinT[:], sin_d[:])
                for c in range(NT):
                    nc.sync.dma_start(wq[c][:], wq_d[c * 128:(c + 1) * 128, :].bitcast(f32r))
                    nc.sync.dma_start(wk[c][:], wk_d[c * 128:(c + 1) * 128, :].bitcast(f32r))
                    nc.sync.dma_start(wv[c][:], wv_d[c * 128:(c + 1) * 128, :].bitcast(f32r))

                for n in range(NCH):
                    ps = [aps.tile([128, 512], f32, tag=f"pa{j}", name=f"pa{j}") for j in range(6)]
                    xc = ax.tile([128, 512], f32r, tag="xc", name="xc")
                    # interleaved accumulation groups over 6 PSUM banks
                    for c in range(NT):
                        xc_c = ax.tile([128, 512], f32r, tag="xc", name="xc") if c else xc
                        nc.sync.dma_start(
                            xc_c[:], xT_d[c * 128:(c + 1) * 128,
                                          n * 512:(n + 1) * 512].bitcast(f32r))
                        for j in range(NH):
                            nc.tensor.matmul(ps[j][:], wq[c][:, j * HD:(j + 1) * HD],
                                             xc_c[:], start=(c == 0), stop=(c == NT - 1))
                        nc.tensor.matmul(ps[4][:], wk[c][:], xc_c[:],
                                         start=(c == 0), stop=(c == NT - 1))
                        nc.tensor.matmul(ps[5][:], wv[c][:], xc_c[:],
                                         start=(c == 0), stop=(c == NT - 1))
                    # RoPE for the 4 Q tiles and K
                    cs, sn = cosT[:, n * 512:(n + 1) * 512], sinT[:, n * 512:(n + 1) * 512]
                    for j in range(5):
                        raw = at.tile([128, 512], f32r, tag="raw", name="raw")
                        nc.vector.tensor_copy(raw[:], ps[j][:])
                        prl = aps.tile([128, 512], f32, tag="prl", name="prl")
                        nc.tensor.matmul(prl[:], shp[:], raw[:], start=True, stop=True)
                        t1 = at.tile([128, 512], f32, tag="t1", name="t1")
                        nc.vector.tensor_tensor(t1[:], raw[:].bitcast(f32), cs, mult)
                        t2 = at.tile([128, 512], f32, tag="t2", name="t2")
                        nc.vector.tensor_tensor(t2[:], prl[:], sn, mult)
                        dst = qt[j][n] if j < NH else kt[n]
                        nc.vector.tensor_tensor(dst[:], t1[:], t2[:], add)
                    # V: PSUM -> SBUF, then PE-transpose 128x128 blocks
                    vf = at.tile([128, 512], f32, tag="vf", name="vf")
                    nc.vector.tensor_copy(vf[:], ps[5][:])
                    for t in range(4):
                        pvt = aps.tile([128, 128], f32, tag="pvt", name="pvt")
                        nc.tensor.transpose(pvt[:], vf[:, t * 128:(t + 1) * 128], idn[:])
                        nc.vector.tensor_copy(vn[n * 4 + t][:], pvt[:])

            # ------------- Phase B: attention, and Phase C: out-projection --
            with tc.tile_pool(name="bexp", bufs=20) as bx, \
                 tc.tile_pool(name="bsm", bufs=3) as bs, \
                 tc.tile_pool(name="yout", bufs=2) as yp, \
                 tc.tile_pool(name="bpsum", bufs=1, space="PSUM") as bps, \
                 tc.tile_pool(name="cpsum", bufs=2, space="PSUM") as cps:
                for qb in range(NCH):
                    for h in range(NH):
                        es = []
                        for k in range(NT):
                            pss = bps.tile([128, 512], f32,
                                           tag=f"pss{k % 3}", name=f"pss{k % 3}")
                            nc.tensor.matmul(pss[:],
                                             kt[k // 4][:, (k % 4) * 128:(k % 4 + 1) * 128],
                                             qt[h][qb][:], start=True, stop=True)
                            e = bx.tile([128, 512], f32r, tag="expS", name="expS")
                            nc.scalar.activation(e[:], pss[:], Exp, scale=SCALE)
                            es.append(e)
                        pso = bps.tile([128, 512], f32, tag="pso", name="pso")
                        for k in range(NT):
                            nc.tensor.matmul(pso[:], vn[k][:], es[k][:],
                                             start=(k == 0), stop=(k == NT - 1))
                        # sumexp: DVE pairwise tree over the 16 exp tiles,
                        # then one ones-matmul for the partition reduction
                        acc = None
                        for i in range(0, NT, 2):
                            s = bs.tile([128, 512], f32r, tag="sacc",
                                        name="sacc", bufs=4)
                            nc.vector.tensor_tensor(
                                s[:], es[i][:].bitcast(f32),
                                es[i + 1][:].bitcast(f32), add)
                            if acc is None:
                                acc = s
                            else:
                                a2 = bs.tile([128, 512], f32r, tag="sacc2",
                                             name="sacc2", bufs=2)
                                nc.vector.tensor_tensor(
                                    a2[:], acc[:].bitcast(f32),
                                    s[:].bitcast(f32), add)
                                acc = a2
                        psr = bps.tile([1, 512], f32, tag="psr", name="psr")
                        nc.tensor.matmul(psr[:], onc[:], acc[:],
                                         start=True, stop=True)
                        sums = bs.tile([1, 512], f32, tag="sums", name="sums")
                        nc.vector.tensor_copy(sums[:], psr[:])
                        rec = bs.tile([1, 512], f32r, tag="rec", name="rec")
                        nc.vector.reciprocal(rec[:], sums[:])
                        prb = bps.tile([128, 512], f32, tag="prb", name="prb")
                        nc.tensor.matmul(prb[:], onr[:], rec[:], start=True, stop=True)
                        rcb = bs.tile([128, 512], f32, tag="rcb", name="rcb")
                        nc.vector.tensor_copy(rcb[:], prb[:])
                        nc.vector.tensor_tensor(ot[h][qb][:], pso[:], rcb[:], mult)
                    # Phase C for the 4 q-tiles covered by this qb
                    for ti in range(4):
                        qtile = qb * 4 + ti
                        ysb = yp.tile([128, L], f32, tag="ysb", name="ysb")
                        for n in range(NCH):
                            psy = cps.tile([128, 512], f32, tag="psy", name="psy")
                            for h in range(NH):
                                nc.tensor.matmul(
                                    psy[:], ot[h][qb][:, ti * 128:(ti + 1) * 128],
                                    wo[h][:, n * 512:(n + 1) * 512],
                                    start=(h == 0), stop=(h == NH - 1))
                            nc.vector.tensor_copy(ysb[:, n * 512:(n + 1) * 512], psy[:])
                        nc.sync.dma_start(
                            y_d[qtile * 128:(qtile + 1) * 128, :], ysb[:])

    nc.compile()
    return nc


def _host_inputs(x, Wq, Wk, Wv, Wo):
    inv = 1.0 / (BASE ** (np.arange(0, HD, 2, dtype=np.float32) / HD))
    pos = np.arange(L, dtype=np.float32)
    fr = pos[:, None] * inv[None, :]
    emb = np.concatenate([fr, fr], axis=1)            # [L, HD]
    cosT = np.ascontiguousarray(np.cos(emb).T)        # [HD, L]
    sinT = np.ascontiguousarray(np.sin(emb).T)
    shp = np.zeros((HD, HD), np.float32)
    shp[(np.arange(HD) - 1) % HD, np.arange(HD)] = 1.0
    idn = np.eye(128, dtype=np.float32)
    onc = np.ones((128, 1), np.float32)
    onr = np.ones((1, 128), np.float32)
    xT = [np.ascontiguousarray(x[b].T) for b in range(B)]
    maps = []
    for c in range(8):
        b, g = c // 4, c % 4
        maps.append({
            "xT": xT[b],
            "wq": np.ascontiguousarray(Wq[:, g * NH * HD:(g + 1) * NH * HD]),
            "wk": np.ascontiguousarray(Wk[:, g * HD:(g + 1) * HD]),
            "wv": np.ascontiguousarray(Wv[:, g * HD:(g + 1) * HD]),
            "wo": np.ascontiguousarray(Wo[g * NH * HD:(g + 1) * NH * HD, :]),
            "cosT": cosT, "sinT": sinT, "shiftP": shp, "ident": idn,
            "onescol": onc, "onesrow": onr,
        })
    return maps


def _run(inputs, trace=False):
    global _compiled
    from concourse.bass_utils import run_bass_kernel_spmd
    if _compiled is None:
        _compiled = _build()
    maps = _host_inputs(inputs["x"], inputs["Wq"], inputs["Wk"],
                        inputs["Wv"], inputs["Wo"])
    res = run_bass_kernel_spmd(_compiled, maps, list(range(8)), trace=trace)
    y = np.empty((B, L, D), np.float32)
    for b in range(B):
        y[b] = res.results[b * 4]["y"]
        for g in range(1, 4):
            y[b] += res.results[b * 4 + g]["y"]
    return y, res


def kernel(**inputs):
    x = np.asarray(inputs["x"], np.float32)
    y, _ = _run({"x": x,
                 "Wq": np.asarray(inputs["Wq"], np.float32),
                 "Wk": np.asarray(inputs["Wk"], np.float32),
                 "Wv": np.asarray(inputs["Wv"], np.float32),
                 "Wo": np.asarray(inputs["Wo"], np.float32)})
    return y


# revision 10
# speedup vs baseline: 24492.8804x; 24492.8804x over previous
"""GQA attention kernel for 8 trn2 NeuronCores.

Sharding: core c in 0..7 -> batch b = c//4, KV group g = c%4 (4 Q heads,
1 KV head per core). Tensor-parallel on Wq/Wk/Wv columns and Wo rows;
host sums the 4 partial outputs per batch.

Layout strategy: all device tensors are transposed ([feature, seq]) so that
Q^T/K^T fall directly out of the projection matmuls, S^T = (K^T-slice)^T @ Q^T
keeps softmax's reduction on the PARTITION axis (summed with a ones-matmul on
the PE), and V (natural [k,d]) is recovered with PE transposes. All matmuls
run in float32r (full PE rate at moving-dim 512, ~1e-4 rel err).
"""
import sys
sys.path.insert(0, "/opt/trn_rl_repo")
import math
import numpy as np

B, L, D = 2, 2048, 2048
H, HKV, HD = 16, 4, 128
BASE = 10000.0
NT = L // 128      # 16 seq tiles of 128
NCH = L // 512     # 4 seq chunks of 512
NH = H // HKV      # 4 heads per core
SCALE = 1.0 / math.sqrt(HD)

_compiled = None


def _build():
    from concourse import bacc, tile, mybir

    f32, f32r = mybir.dt.float32, mybir.dt.float32r
    Exp = mybir.ActivationFunctionType.Exp
    mult, add = mybir.AluOpType.mult, mybir.AluOpType.add

    nc = bacc.Bacc("TRN2", target_bir_lowering=False, debug=False,
                   enable_asserts=True, num_devices=8)

    xT_d = nc.dram_tensor("xT", [D, L], f32, kind="ExternalInput")
    wq_d = nc.dram_tensor("wq", [D, NH * HD], f32, kind="ExternalInput")
    wk_d = nc.dram_tensor("wk", [D, HD], f32, kind="ExternalInput")
    wv_d = nc.dram_tensor("wv", [D, HD], f32, kind="ExternalInput")
    wo_d = nc.dram_tensor("wo", [NH * HD, D], f32, kind="ExternalInput")
    cos_d = nc.dram_tensor("cosT", [HD, L], f32, kind="ExternalInput")
    sin_d = nc.dram_tensor("sinT", [HD, L], f32, kind="ExternalInput")
    shp_d = nc.dram_tensor("shiftP", [HD, HD], f32, kind="ExternalInput")
    idn_d = nc.dram_tensor("ident", [128, 128], f32, kind="ExternalInput")
    onc_d = nc.dram_tensor("onescol", [128, 1], f32, kind="ExternalInput")
    onr_d = nc.dram_tensor("onesrow", [1, 128], f32, kind="ExternalInput")
    y_d = nc.dram_tensor("y", [L, D], f32, kind="ExternalOutput")

    with tile.TileContext(nc) as tc, \
         nc.allow_low_precision(reason="float32r is 4-byte storage; matmul "
                                "inputs round to fp32r by design"):
        with tc.tile_pool(name="persist", bufs=1) as pp:
            # long-lived tiles (unique tag per tile => exact allocation)
            shp = pp.tile([HD, HD], f32r, tag="shp", name="shp")
            idn = pp.tile([128, 128], f32, tag="idn", name="idn")
            onc = pp.tile([128, 1], f32r, tag="onc", name="onc")
            onr = pp.tile([1, 128], f32r, tag="onr", name="onr")
            nc.sync.dma_start(shp[:], shp_d[:].bitcast(f32r))
            nc.sync.dma_start(idn[:], idn_d[:])
            nc.sync.dma_start(onc[:], onc_d[:].bitcast(f32r))
            nc.sync.dma_start(onr[:], onr_d[:].bitcast(f32r))

            qt = [[pp.tile([HD, 512], f32r, tag=f"qt{h}_{n}", name=f"qt{h}_{n}") for n in range(NCH)]
                  for h in range(NH)]
            kt = [pp.tile([HD, 512], f32r, tag=f"kt{n}", name=f"kt{n}") for n in range(NCH)]
            vn = [pp.tile([128, HD], f32r, tag=f"vn{t}", name=f"vn{t}") for t in range(NT)]
            ot = [[pp.tile([HD, 512], f32r, tag=f"ot{h}_{n}", name=f"ot{h}_{n}") for n in range(NCH)]
                  for h in range(NH)]
            wo = [pp.tile([HD, L], f32r, tag=f"wo{h}", name=f"wo{h}") for h in range(NH)]

            # ---------------- Phase A: projections + RoPE + V transpose ----
            with tc.tile_pool(name="aw", bufs=1) as aw, \
                 tc.tile_pool(name="ax", bufs=3) as ax, \
                 tc.tile_pool(name="atmp", bufs=3) as at, \
                 tc.tile_pool(name="apsum", bufs=1, space="PSUM") as aps:
                wq = [aw.tile([128, NH * HD], f32r, tag=f"wq{c}", name=f"wq{c}") for c in range(NT)]
                wk = [aw.tile([128, HD], f32r, tag=f"wk{c}", name=f"wk{c}") for c in range(NT)]
                wv = [aw.tile([128, HD], f32r, tag=f"wv{c}", name=f"wv{c}") for c in range(NT)]
                cosT = aw.tile([HD, L], f32, tag="cos", name="cos")
                sinT = aw.tile([HD, L], f32, tag="sin", name="sin")
                nc.sync.dma_start(cosT[:], cos_d[:])
                nc.sync.dma_start(sinT[:], sin_d[:])
                for n in range(NCH):
                    ps = [aps.tile([128, 512], f32, tag=f"pa{j}", name=f"pa{j}") for j in range(6)]
                    xc = ax.tile([128, 512], f32r, tag="xc", name="xc")
                    # interleaved accumulation groups over 6 PSUM banks
                    for c in range(NT):
                        xc_c = ax.tile([128, 512], f32r, tag="xc", name="xc") if c else xc
                        nc.sync.dma_start(
                            xc_c[:], xT_d[c * 128:(c + 1) * 128,
                                          n * 512:(n + 1) * 512].bitcast(f32r))
                        if n == 0:
                            nc.sync.dma_start(wq[c][:], wq_d[c * 128:(c + 1) * 128, :].bitcast(f32r))
                            nc.sync.dma_start(wk[c][:], wk_d[c * 128:(c + 1) * 128, :].bitcast(f32r))
                            nc.sync.dma_start(wv[c][:], wv_d[c * 128:(c + 1) * 128, :].bitcast(f32r))
                        for j in range(NH):
                            nc.tensor.matmul(ps[j][:], wq[c][:, j * HD:(j + 1) * HD],
                                             xc_c[:], start=(c == 0), stop=(c == NT - 1))
                        nc.tensor.matmul(ps[4][:], wk[c][:], xc_c[:],
                                         start=(c == 0), stop=(c == NT - 1))
                        nc.tensor.matmul(ps[5][:], wv[c][:], xc_c[:],
                                         start=(c == 0), stop=(c == NT - 1))
                    # RoPE for the 4 Q tiles and K
                    cs, sn = cosT[:, n * 512:(n + 1) * 512], sinT[:, n * 512:(n + 1) * 512]
                    for j in range(5):
                        raw = at.tile([128, 512], f32r, tag="raw", name="raw")
                        nc.vector.tensor_copy(raw[:], ps[j][:])
                        prl = aps.tile([128, 512], f32, tag="prl", name="prl")
                        nc.tensor.matmul(prl[:], shp[:], raw[:], start=True, stop=True)
                        t1 = at.tile([128, 512], f32, tag="t1", name="t1")
                        nc.vector.tensor_tensor(t1[:], raw[:].bitcast(f32), cs, mult)
                        t2 = at.tile([128, 512], f32, tag="t2", name="t2")
                        nc.vector.tensor_tensor(t2[:], prl[:], sn, mult)
                        dst = qt[j][n] if j < NH else kt[n]
                        nc.vector.tensor_tensor(dst[:], t1[:], t2[:], add)
                    # V: PSUM -> SBUF, then PE-transpose 128x128 blocks
                    vf = at.tile([128, 512], f32, tag="vf", name="vf")
                    nc.vector.tensor_copy(vf[:], ps[5][:])
                    for t in range(4):
                        pvt = aps.tile([128, 128], f32, tag="pvt", name="pvt")
                        nc.tensor.transpose(pvt[:], vf[:, t * 128:(t + 1) * 128], idn[:])
                        nc.vector.tensor_copy(vn[n * 4 + t][:], pvt[:])

            # ------------- Phase B: attention, and Phase C: out-projection --
            with tc.tile_pool(name="bexp", bufs=20) as bx, \
                 tc.tile_pool(name="bsm", bufs=3) as bs, \
                 tc.tile_pool(name="yout", bufs=2) as yp, \
                 tc.tile_pool(name="bpsum", bufs=1, space="PSUM") as bps, \
                 tc.tile_pool(name="cpsum", bufs=2, space="PSUM") as cps:
                for h in range(NH):
                    nc.sync.dma_start(wo[h][:], wo_d[h * HD:(h + 1) * HD, :].bitcast(f32r))
                for qb in range(NCH):
                    for h in range(NH):
                        es = []
                        for k in range(NT):
                            pss = bps.tile([128, 512], f32,
                                           tag=f"pss{k % 2}", name=f"pss{k % 2}")
                            nc.tensor.matmul(pss[:],
                                             kt[k // 4][:, (k % 4) * 128:(k % 4 + 1) * 128],
                                             qt[h][qb][:], start=True, stop=True)
                            e = bx.tile([128, 512], f32r, tag="expS", name="expS")
                            nc.scalar.activation(e[:], pss[:], Exp, scale=SCALE)
                            es.append(e)
                        pso = bps.tile([128, 512], f32, tag="pso", name="pso",
                                       bufs=2)
                        psr = bps.tile([1, 512], f32, tag="psr", name="psr")
                        for k in range(NT):
                            nc.tensor.matmul(pso[:], vn[k][:], es[k][:],
                                             start=(k == 0), stop=(k == NT - 1))
                        # sumexp: one DVE pairwise level, then 8 ones-matmuls
                        for i in range(0, NT, 2):
                            s = bs.tile([128, 512], f32r, tag="sacc",
                                        name="sacc", bufs=3)
                            nc.vector.tensor_tensor(
                                s[:], es[i][:].bitcast(f32),
                                es[i + 1][:].bitcast(f32), add)
                            nc.tensor.matmul(psr[:], onc[:], s[:],
                                             start=(i == 0), stop=(i == NT - 2))
                        sums = bs.tile([1, 512], f32, tag="sums", name="sums")
                        nc.vector.tensor_copy(sums[:], psr[:])
                        rec = bs.tile([1, 512], f32r, tag="rec", name="rec")
                        nc.vector.reciprocal(rec[:], sums[:])
                        prb = bps.tile([128, 512], f32, tag="prb", name="prb")
                        nc.tensor.matmul(prb[:], onr[:], rec[:], start=True, stop=True)
                        rcb = bs.tile([128, 512], f32, tag="rcb", name="rcb")
                        nc.vector.tensor_copy(rcb[:], prb[:])
                        nc.vector.tensor_tensor(ot[h][qb][:], pso[:], rcb[:], mult)
                    # Phase C for the 4 q-tiles covered by this qb
                    for ti in range(4):
                        qtile = qb * 4 + ti
                        ysb = yp.tile([128, L], f32, tag="ysb", name="ysb")
                        for n in range(NCH):
                            psy = cps.tile([128, 512], f32, tag="psy", name="psy")
                            for h in range(NH):
                                nc.tensor.matmul(
                                    psy[:], ot[h][qb][:, ti * 128:(ti + 1) * 128],
                                    wo[h][:, n * 512:(n + 1) * 512],
                                    start=(h == 0), stop=(h == NH - 1))
                            nc.scalar.activation(ysb[:, n * 512:(n + 1) * 512], psy[:],
                                                 mybir.ActivationFunctionType.Copy)
                        nc.sync.dma_start(
                            y_d[qtile * 128:(qtile + 1) * 128, :], ysb[:])

    nc.compile()
    return nc


def _host_inputs(x, Wq, Wk, Wv, Wo):
    inv = 1.0 / (BASE ** (np.arange(0, HD, 2, dtype=np.float32) / HD))
    pos = np.arange(L, dtype=np.float32)
    fr = pos[:, None] * inv[None, :]
    emb = np.concatenate([fr, fr], axis=1)            # [L, HD]
    cosT = np.ascontiguousarray(np.cos(emb).T)        # [HD, L]
    sinT = np.ascontiguousarray(np.sin(emb).T)
    shp = np.zeros((HD, HD), np.float32)
    shp[(np.arange(HD) - 1) % HD, np.arange(HD)] = 1.0
    idn = np.eye(128, dtype=np.float32)
    onc = np.ones((128, 1), np.float32)
    onr = np.ones((1, 128), np.float32)
    xT = [np.ascontiguousarray(x[b].T) for b in range(B)]
    maps = []
    for c in range(8):
        b, g = c // 4, c % 4
        maps.append({
            "xT": xT[b],
            "wq": np.ascontiguousarray(Wq[:, g * NH * HD:(g + 1) * NH * HD]),
            "wk": np.ascontiguousarray(Wk[:, g * HD:(g + 1) * HD]),
            "wv": np.ascontiguousarray(Wv[:, g * HD:(g + 1) * HD]),
            "wo": np.ascontiguousarray(Wo[g * NH * HD:(g + 1) * NH * HD, :]),
            "cosT": cosT, "sinT": sinT, "shiftP": shp, "ident": idn,
            "onescol": onc, "onesrow": onr,
        })
    return maps


def _run(inputs, trace=False):
    global _compiled
    from concourse.bass_utils import run_bass_kernel_spmd
    if _compiled is None:
        _compiled = _build()
    maps = _host_inputs(inputs["x"], inputs["Wq"], inputs["Wk"],
                        inputs["Wv"], inputs["Wo"])
    res = run_bass_kernel_spmd(_compiled, maps, list(range(8)), trace=trace)
    y = np.empty((B, L, D), np.float32)
    for b in range(B):
        y[b] = res.results[b * 4]["y"]
        for g in range(1, 4):
            y[b] += res.results[b * 4 + g]["y"]
    return y, res


def kernel(**inputs):
    x = np.asarray(inputs["x"], np.float32)
    y, _ = _run({"x": x,
                 "Wq": np.asarray(inputs["Wq"], np.float32),
                 "Wk": np.asarray(inputs["Wk"], np.float32),
                 "Wv": np.asarray(inputs["Wv"], np.float32),
                 "Wo": np.asarray(inputs["Wo"], np.float32)})
    return y
